# revision 27
# baseline (speedup 1.0000x reference)
"""Causal multi-head attention on 8 TRN2 NeuronCores.

Problem: B=4, S=2048, D=1024, H=16, HD=64, causal MHA with out-proj + bias.

Sharding: core c handles (batch b = c//2, head-half hh = c%2), i.e. 8 heads of
one batch element. Per core:
  Q^T/K^T = (Wq/Wk half)^T X_b^T   -> [64*2, S] per head pair (head on partition)
  V       = X_b @ Wv half          -> [S, 8*65] (65 = 64 + fused-ones column)
  S^T_j   = K_j Q^T (scores transposed: keys on partition) per 128-key block
  P^T     = exp(S^T/8) (ScalarE), causal handled by block skipping + a
            multiplicative 0/1 bf16 mask applied AFTER exp on GpSimd (keeps
            the DVE queue off the attention critical path)
  ctx'^T  = V'^T P^T accumulated over key blocks; row 64 = softmax denominators
            (ones-column trick)
  ctxT    = ctx'^T * (1/denom) broadcast  (GpSimd partition_broadcast)
  out     = ctxT^T @ Wo half  (partial; host sums the two half partials + bias)

Scheduling (the perf-critical part):
  - Inputs arrive in ~11 large host-packed DMAs ordered so the first
    projection can start ~8us in (each dma_start costs ~600ns serial on SP).
  - The attention inner loop is software-pipelined: scores(j+1) is issued
    before PV(j), so the PE never waits for the ACT-engine exp. Projection /
    out-proj matmuls are interleaved as filler between steps to keep the PE
    gapless (which also holds it at the 2.4GHz p-state).
  - Even/odd-head score tiles share one 2-bank PSUM tile [128,1024] so a
    single ACT instruction exps both, halving ACT per-instruction bubbles.

All matmuls in bf16 (fp32 accumulate in PSUM); scores/softmax in fp32.
"""

import numpy as np
import ml_dtypes
from contextlib import ExitStack

import concourse.bass as bass
import concourse.bacc as bacc
import concourse.mybir as mybir
import concourse.tile as tile
from concourse import bass_utils

F32 = mybir.dt.float32
BF16 = mybir.dt.bfloat16

B, S, D = 4, 2048, 1024
H, HD = 16, 64
DH = 512          # columns of the head-half handled by one core (8 heads * 64)
NCORES = 8
CH = 512          # q chunk width
NCH = S // CH     # 4
NKB = S // 128    # 16 key/query 128-blocks
KT = D // 128     # 8 contraction tiles for the projections
NWARM = 24        # PE warm-up matmuls issued while input DMAs land

_CACHED = None


def build_module():
    nc = bacc.Bacc("TRN2", target_bir_lowering=False, debug=False)

    # --- dram tensors: host-packed for few, large, contiguous DMAs --------
    # x chunk-major: xc{c}[p, 512*k + s] = X^T[128k+p, 512c+s]
    xcd = [nc.dram_tensor(f"xc{c}", [128, KT * CH], BF16, kind="ExternalInput")
           for c in range(NCH)]
    # wk/wq pair-major, split so pair 0 can arrive first:
    #   wk0[p, 128k+i] = Wk[128k+p, i]  (pair g=0)
    #   wkR[p, 1024(g-1) + 128k + i] = Wk[128k+p, 128g+i]  (pairs 1..3)
    wk0d = nc.dram_tensor("wk0", [128, KT * 128], BF16, kind="ExternalInput")
    wq0d = nc.dram_tensor("wq0", [128, KT * 128], BF16, kind="ExternalInput")
    wkRd = nc.dram_tensor("wkR", [128, 3 * KT * 128], BF16, kind="ExternalInput")
    wqRd = nc.dram_tensor("wqR", [128, 3 * KT * 128], BF16, kind="ExternalInput")
    # wv k-major: wv[p, 512k+j] = Wv[128k+p, j]
    wvd = nc.dram_tensor("wv", [128, KT * DH], BF16, kind="ExternalInput")
    # wo g-major: wo[p, 1024g+j] = Wo[128g+p, j]
    wod = nc.dram_tensor("wo", [128, 4 * D], BF16, kind="ExternalInput")
    maskt = nc.dram_tensor("maskt", [128, 128], BF16, kind="ExternalInput")
    # bf16 partials: halves the output DMA; the host sums the two half
    # partials in fp32 (quantization adds ~2e-3 rel err, well within budget)
    out = nc.dram_tensor("out", [S, D], BF16, kind="ExternalOutput")

    with tile.TileContext(nc) as tc, ExitStack() as ctx:
        const = ctx.enter_context(tc.tile_pool(name="const", bufs=1))
        xTp = ctx.enter_context(tc.tile_pool(name="xTp", bufs=1))
        wp = ctx.enter_context(tc.tile_pool(name="wp", bufs=1))
        qkp = ctx.enter_context(tc.tile_pool(name="qkp", bufs=1))
        vp = ctx.enter_context(tc.tile_pool(name="vp", bufs=1))
        ctp = ctx.enter_context(tc.tile_pool(name="ctp", bufs=1))
        pTp = ctx.enter_context(tc.tile_pool(name="pTp", bufs=4))
        rp = ctx.enter_context(tc.tile_pool(name="rp", bufs=3))
        bp = ctx.enter_context(tc.tile_pool(name="bp", bufs=4))
        osb = ctx.enter_context(tc.tile_pool(name="osb", bufs=2))
        ps_sc = ctx.enter_context(tc.tile_pool(name="ps_sc", bufs=2, space="PSUM"))
        ps_ctx = ctx.enter_context(tc.tile_pool(name="ps_ctx", bufs=2, space="PSUM"))
        ps_mm = ctx.enter_context(tc.tile_pool(name="ps_mm", bufs=2, space="PSUM"))

        # --- SBUF persistents --------------------------------------------
        xbig = xTp.tile([128, KT * S], BF16, name="xbig", tag="xbig")
        wk0 = wp.tile([128, KT * 128], BF16, name="wk0", tag="wk0")
        wq0 = wp.tile([128, KT * 128], BF16, name="wq0", tag="wq0")
        wkR = wp.tile([128, 3 * KT * 128], BF16, name="wkR", tag="wkR")
        wqR = wp.tile([128, 3 * KT * 128], BF16, name="wqR", tag="wqR")
        wv = wp.tile([128, KT * DH], BF16, name="wv", tag="wv")
        wo = wp.tile([128, 4 * D], BF16, name="wo", tag="wo")
        mask = const.tile([128, 128], BF16, name="mask", tag="mask")
        scr = const.tile([128, CH], BF16, name="scr", tag="scr")
        ones8 = const.tile([128, 8], BF16, name="ones8", tag="ones8")

        def xts(k):
            return xbig[:, S * k:S * (k + 1)]

        def wk_sl(g, k):
            if g == 0:
                return wk0[:, 128 * k:128 * (k + 1)]
            return wkR[:, 1024 * (g - 1) + 128 * k:1024 * (g - 1) + 128 * (k + 1)]

        def wq_sl(g, k):
            if g == 0:
                return wq0[:, 128 * k:128 * (k + 1)]
            return wqR[:, 1024 * (g - 1) + 128 * k:1024 * (g - 1) + 128 * (k + 1)]

        # --- input DMAs, priority order ----------------------------------
        nc.sync.dma_start(wk0[:], wk0d[:])
        nc.sync.dma_start(wq0[:], wq0d[:])
        xr = xbig[:].rearrange("p (k s) -> p k s", s=S)
        xcr0 = xcd[0][:].rearrange("p (k s) -> p k s", s=CH)
        nc.sync.dma_start(xr[:, 0:4, 0:CH], xcr0[:, 0:4, :])
        nc.sync.dma_start(xr[:, 4:KT, 0:CH], xcr0[:, 4:KT, :])
        nc.sync.dma_start(mask[:], maskt[:])
        nc.sync.dma_start(wv[:], wvd[:])
        for c in range(1, NCH):
            nc.sync.dma_start(xr[:, :, CH * c:CH * (c + 1)], xcd[c][:])
        nc.sync.dma_start(wkR[:], wkRd[:])
        nc.sync.dma_start(wqR[:], wqRd[:])
        nc.sync.dma_start(wo[:], wod[:])

        # --- PE warm-up: junk matmuls on a memset tile keep the PE busy
        # (and its p-state ramping) while the first inputs stream in.
        nc.vector.memset(scr[:], 0.0)
        nc.vector.memset(ones8[:], 1.0)
        wps = ps_mm.tile([128, CH], F32, name="mm", tag="mm")
        for _ in range(NWARM):
            nc.tensor.matmul(wps[:], lhsT=scr[:, 0:128], rhs=scr[:],
                             skip_group_check=True)

        # --- persistent intermediates ------------------------------------
        qts = [qkp.tile([128, S], BF16, name=f"qt{g}", tag=f"qt{g}") for g in range(4)]
        kts = [qkp.tile([128, S], BF16, name=f"kt{g}", tag=f"kt{g}") for g in range(4)]
        vts = [vp.tile([128, 8 * 65], BF16, name=f"v{m}", tag=f"v{m}") for m in range(NKB)]
        cts = [ctp.tile([128, S], BF16, name=f"ct{g}", tag=f"ct{g}") for g in range(4)]
        # the fused-ones columns of every V tile are constant: set them once
        for m in range(NKB):
            ones = vts[m][:].rearrange("p (h x) -> p h x", x=65)[:, :, 64:65]
            nc.vector.tensor_copy(ones, ones8[:])

        # --- filler units: closures over in-flight PSUM accumulations ----
        fillers = []
        fill_pos = 0

        def pop_fill(n):
            nonlocal fill_pos
            end = min(fill_pos + n, len(fillers))
            while fill_pos < end:
                fillers[fill_pos][1]()
                fill_pos += 1

        def drain_tag(tag):
            """Run every queued unit up to and including the last one with
            this tag (correctness-of-perf: deps are real semaphores)."""
            nonlocal fill_pos
            last = -1
            for i in range(fill_pos, len(fillers)):
                if fillers[i][0] == tag:
                    last = i
            while fill_pos <= last:
                fillers[fill_pos][1]()
                fill_pos += 1

        def push_qk_units(dst, wsl, g, c):
            st8 = {}

            def a():
                ps = ps_mm.tile([128, CH], F32, name="mm", tag="mm")
                st8["ps"] = ps
                for k in range(4):
                    nc.tensor.matmul(ps[:], lhsT=wsl(g, k),
                                     rhs=xts(k)[:, CH * c:CH * (c + 1)],
                                     start=(k == 0), stop=False,
                                     skip_group_check=True)

            def b():
                ps = st8["ps"]
                for k in range(4, KT):
                    nc.tensor.matmul(ps[:], lhsT=wsl(g, k),
                                     rhs=xts(k)[:, CH * c:CH * (c + 1)],
                                     start=False, stop=(k == KT - 1),
                                     skip_group_check=True)
                nc.vector.tensor_copy(dst[:, CH * c:CH * (c + 1)], ps[:])

            tag = f"qk{g}c{c}"
            fillers.append((tag, a))
            fillers.append((tag, b))

        def push_v_units(m):
            st8 = {}

            def a():
                ps = ps_mm.tile([128, CH], F32, name="mm", tag="mm")
                st8["ps"] = ps
                for k in range(4):
                    nc.tensor.matmul(ps[:], lhsT=xts(k)[:, 128 * m:128 * (m + 1)],
                                     rhs=wv[:, DH * k:DH * (k + 1)],
                                     start=(k == 0), stop=False,
                                     skip_group_check=True)

            def b():
                ps = st8["ps"]
                for k in range(4, KT):
                    nc.tensor.matmul(ps[:], lhsT=xts(k)[:, 128 * m:128 * (m + 1)],
                                     rhs=wv[:, DH * k:DH * (k + 1)],
                                     start=False, stop=(k == KT - 1),
                                     skip_group_check=True)
                vm = vts[m]
                dst = vm[:].rearrange("p (h x) -> p h x", x=65)[:, :, 0:64]
                src = ps[:].rearrange("p (h d) -> p h d", d=64)
                nc.vector.tensor_copy(dst, src)

            tag = f"v{m}"
            fillers.append((tag, a))
            fillers.append((tag, b))

        def push_outproj_units(c):
            for qb in range(4 * c, 4 * c + 4):
                st8 = {}

                def a(qb=qb, st8=st8):
                    ost = osb.tile([128, D], BF16, name="ost", tag="ost")
                    ps = ps_mm.tile([128, CH], F32, name="mm", tag="mm")
                    st8["ost"], st8["ps"] = ost, ps
                    for g in range(4):
                        nc.tensor.matmul(ps[:], lhsT=cts[g][:, 128 * qb:128 * (qb + 1)],
                                         rhs=wo[:, D * g:D * g + CH],
                                         start=(g == 0), stop=(g == 3),
                                         skip_group_check=True)
                    nc.vector.tensor_copy(ost[:, 0:CH], ps[:])

                def b(qb=qb, st8=st8):
                    ost = st8["ost"]
                    ps = ps_mm.tile([128, CH], F32, name="mm", tag="mm")
                    for g in range(4):
                        nc.tensor.matmul(ps[:], lhsT=cts[g][:, 128 * qb:128 * (qb + 1)],
                                         rhs=wo[:, D * g + CH:D * (g + 1)],
                                         start=(g == 0), stop=(g == 3),
                                         skip_group_check=True)
                    nc.vector.tensor_copy(ost[:, CH:D], ps[:])
                    nc.sync.dma_start(out[128 * qb:128 * (qb + 1), :], ost[:])

                tag = f"op{c}"
                fillers.append((tag, a))
                fillers.append((tag, b))

        # --- attention ----------------------------------------------------
        def run_chunk(g, c):
            """Heads (2g, 2g+1), queries [CH*c, CH*(c+1)); software-pipelined:
            sc(j+1) and filler units are issued before pv(j)."""
            drain_tag(f"qk{g}c{c}")
            qt, ktile = qts[g], kts[g]
            nj = 4 * c + 4
            ctx_e = ps_ctx.tile([65, CH], F32, name="ctx", tag="ctx")
            ctx_o = ps_ctx.tile([65, CH], F32, name="ctx", tag="ctx")
            pts = [None] * nj

            def sc_step(j):
                d = j - 4 * c
                st = 128 * max(0, d)
                sc = ps_sc.tile([128, 2 * CH], F32, name="sc", tag="sc")
                # even head in bank 0 (cols 0:512), odd head in bank 1
                nc.tensor.matmul(
                    sc[:, st:CH], lhsT=ktile[0:64, 128 * j:128 * (j + 1)],
                    rhs=qt[0:64, CH * c + st:CH * (c + 1)])
                nc.tensor.matmul(
                    sc[:, CH + st:2 * CH], lhsT=ktile[64:128, 128 * j:128 * (j + 1)],
                    rhs=qt[64:128, CH * c + st:CH * (c + 1)])
                pt = pTp.tile([128, 2 * CH], BF16, name="pT", tag="pT")
                if st == 0:
                    # one exp covers both heads
                    nc.scalar.activation(pt[:], sc[:],
                                         mybir.ActivationFunctionType.Exp,
                                         scale=0.125)
                else:
                    nc.scalar.activation(pt[:, st:CH], sc[:, st:CH],
                                         mybir.ActivationFunctionType.Exp,
                                         scale=0.125)
                    nc.scalar.activation(pt[:, CH + st:], sc[:, CH + st:],
                                         mybir.ActivationFunctionType.Exp,
                                         scale=0.125)
                if d >= 0:
                    # causal zeroing of the diagonal square, post-exp: a cheap
                    # bf16 SBUF multiply on DVE (GpSimd must stay single-ucode
                    # for partition_broadcast — mixing op kinds there forces
                    # ~6us LOAD_LIB swaps)
                    ms = slice(st, st + 128)
                    nc.vector.tensor_tensor(pt[:, ms], pt[:, ms], mask[:],
                                            op=mybir.AluOpType.mult)
                    ms2 = slice(CH + st, CH + st + 128)
                    nc.vector.tensor_tensor(pt[:, ms2], pt[:, ms2], mask[:],
                                            op=mybir.AluOpType.mult)
                pts[j] = (pt, st)

            def pv_step(j):
                pt, st = pts[j]
                he, ho = 2 * g, 2 * g + 1
                nc.tensor.matmul(ctx_e[:, st:], lhsT=vts[j][:, 65 * he:65 * he + 65],
                                 rhs=pt[:, st:CH], start=(j == 0), stop=(j == nj - 1),
                                 skip_group_check=True)
                nc.tensor.matmul(ctx_o[:, st:], lhsT=vts[j][:, 65 * ho:65 * ho + 65],
                                 rhs=pt[:, CH + st:2 * CH], start=(j == 0),
                                 stop=(j == nj - 1), skip_group_check=True)

            sc_step(0)
            for j in range(nj):
                if j + 1 < nj:
                    drain_tag(f"v{j + 1}")  # usually a no-op; keeps pv fed
                    pop_fill(1)  # spaces sc(j) and sc(j+1) PSUM-bank reuse
                    sc_step(j + 1)
                pop_fill(1)
                pv_step(j)

            # normalize by the fused denominator row and store into ctxT (bf16)
            for ctx_ps, rows in ((ctx_e, slice(0, 64)), (ctx_o, slice(64, 128))):
                # copy the sums row to partition 0 (both the custom-DVE recip
                # and the HW partition_broadcast need a partition-0 source),
                # take the reciprocal, and broadcast it across 64 partitions
                srow = rp.tile([1, CH], F32, name="srow", tag="srow")
                nc.vector.tensor_copy(srow[:], ctx_ps[64:65, :])
                rc = rp.tile([1, CH], F32, name="recip", tag="recip")
                nc.vector.reciprocal_approx_fast(rc[:], srow[:])
                bc = bp.tile([64, CH], F32, name="bcast", tag="bcast")
                nc.gpsimd.partition_broadcast(bc[:], rc[:])

                # the cts write is deferred into the filler queue (popped early
                # in the next chunk, before its first pv reuses the ctx bank)
                # so the DVE queue never blocks the next chunk's
                # mask -> exp -> PV chain on this chunk's normalize
                def mult(ctx_ps=ctx_ps, bc=bc, rows=rows, g=g, c=c):
                    nc.vector.tensor_tensor(cts[g][rows, CH * c:CH * (c + 1)],
                                            ctx_ps[0:64, :], bc[:],
                                            op=mybir.AluOpType.mult)
                fillers.insert(fill_pos, ("norm", mult))

        # --- schedule -----------------------------------------------------
        # pair-0 K/Q chunk 0 directly (first PE work, ~8us in)
        push_qk_units(kts[0], wk_sl, 0, 0)
        push_qk_units(qts[0], wq_sl, 0, 0)
        drain_tag("qk0c0")
        # filler queue for the g=0 phase: V blocks as needed + later chunks
        for m in range(4):
            push_v_units(m)
        for c in range(1, NCH):
            push_qk_units(kts[0], wk_sl, 0, c)
            push_qk_units(qts[0], wq_sl, 0, c)
            for m in range(4 * c, 4 * c + 4):
                push_v_units(m)
        # pair-1 projections also as g=0-phase filler
        for c in range(NCH):
            push_qk_units(kts[1], wk_sl, 1, c)
            push_qk_units(qts[1], wq_sl, 1, c)

        for g in range(4):
            if g >= 1:
                # next pair's projections pop as filler during this phase
                for c in range(NCH):
                    if g + 1 < 4:
                        push_qk_units(kts[g + 1], wk_sl, g + 1, c)
                        push_qk_units(qts[g + 1], wq_sl, g + 1, c)
            for c in range(NCH):
                run_chunk(g, c)
                if g == 3:
                    push_outproj_units(c)
        pop_fill(len(fillers))  # flush remaining out-proj units

    nc.compile()
    return nc


def _get_module():
    global _CACHED
    if _CACHED is None:
        _CACHED = build_module()
    return _CACHED


def _causal_mask_tile():
    k = np.arange(128)[:, None]
    q = np.arange(128)[None, :]
    return np.where(k <= q, 1.0, 0.0).astype(ml_dtypes.bfloat16)


def make_in_maps(inputs, Wq, Wk, Wv, Wo):
    """Host-side packing into the dram-tensor layouts (see build_module)."""
    bf = ml_dtypes.bfloat16
    mask = _causal_mask_tile()
    in_maps = []
    for core in range(NCORES):
        b, hh = core // 2, core % 2
        cols = slice(DH * hh, DH * (hh + 1))
        wqh = Wq[:, cols].reshape(KT, 128, 4, 128)   # [k,p,g,i]
        wkh = Wk[:, cols].reshape(KT, 128, 4, 128)
        wvh = Wv[:, cols].reshape(KT, 128, DH)       # [k,p,j]
        woh = Wo[cols, :].reshape(4, 128, D)         # [g,p,j]
        xT = inputs[b].T.reshape(KT, 128, NCH, CH)   # [k,p,c,s]
        m = {
            "wk0": wkh[:, :, 0, :].transpose(1, 0, 2).reshape(128, KT * 128),
            "wq0": wqh[:, :, 0, :].transpose(1, 0, 2).reshape(128, KT * 128),
            "wkR": wkh[:, :, 1:, :].transpose(1, 2, 0, 3).reshape(128, 3 * KT * 128),
            "wqR": wqh[:, :, 1:, :].transpose(1, 2, 0, 3).reshape(128, 3 * KT * 128),
            "wv": wvh.transpose(1, 0, 2).reshape(128, KT * DH),
            "wo": woh.transpose(1, 0, 2).reshape(128, 4 * D),
        }
        for c in range(NCH):
            m[f"xc{c}"] = xT[:, :, c, :].transpose(1, 0, 2).reshape(128, KT * CH)
        m = {k: np.ascontiguousarray(v).astype(bf) for k, v in m.items()}
        m["maskt"] = mask
        in_maps.append(m)
    return in_maps


def kernel(inputs, Wq, Wk, Wv, Wo, bo):
    inputs = np.asarray(inputs, dtype=np.float32)
    Wq = np.asarray(Wq, dtype=np.float32)
    Wk = np.asarray(Wk, dtype=np.float32)
    Wv = np.asarray(Wv, dtype=np.float32)
    Wo = np.asarray(Wo, dtype=np.float32)
    bo = np.asarray(bo, dtype=np.float32)

    in_maps = make_in_maps(inputs, Wq, Wk, Wv, Wo)
    nc = _get_module()
    res = bass_utils.run_bass_kernel_spmd(nc, in_maps, core_ids=list(range(NCORES)))
    outs = [r["out"] for r in res.results]

    full = np.empty((B, S, D), dtype=np.float32)
    for b in range(B):
        full[b] = (outs[2 * b].astype(np.float32)
                   + outs[2 * b + 1].astype(np.float32) + bo[None, :])
    return full


# revision 29
# speedup vs baseline: 1.0502x; 1.0502x over previous
"""Causal multi-head attention on 8 TRN2 NeuronCores.

Problem: B=4, S=2048, D=1024, H=16, HD=64, causal MHA with out-proj + bias.

Sharding: core c handles (batch b = c//2, head-half hh = c%2), i.e. 8 heads of
one batch element. Per core:
  Q^T/K^T = (Wq/Wk half)^T X_b^T   -> [64*2, S] per head pair (head on partition)
  V       = X_b @ Wv half          -> [S, 8*65] (65 = 64 + fused-ones column)
  S^T_j   = K_j Q^T (scores transposed: keys on partition) per 128-key block
  P^T     = exp(S^T/8) (ScalarE), causal handled by block skipping + a
            multiplicative 0/1 bf16 mask applied AFTER exp on GpSimd (keeps
            the DVE queue off the attention critical path)
  ctx'^T  = V'^T P^T accumulated over key blocks; row 64 = softmax denominators
            (ones-column trick)
  ctxT    = ctx'^T * (1/denom) broadcast  (GpSimd partition_broadcast)
  out     = ctxT^T @ Wo half  (partial; host sums the two half partials + bias)

Scheduling (the perf-critical part):
  - Inputs arrive in ~11 large host-packed DMAs ordered so the first
    projection can start ~8us in (each dma_start costs ~600ns serial on SP).
  - The attention inner loop is software-pipelined: scores(j+1) is issued
    before PV(j), so the PE never waits for the ACT-engine exp. Projection /
    out-proj matmuls are interleaved as filler between steps to keep the PE
    gapless (which also holds it at the 2.4GHz p-state).
  - Even/odd-head score tiles share one 2-bank PSUM tile [128,1024] so a
    single ACT instruction exps both, halving ACT per-instruction bubbles.

All matmuls in bf16 (fp32 accumulate in PSUM); scores/softmax in fp32.
"""

import numpy as np
import ml_dtypes
from contextlib import ExitStack

import concourse.bass as bass
import concourse.bacc as bacc
import concourse.mybir as mybir
import concourse.tile as tile
from concourse import bass_utils

F32 = mybir.dt.float32
BF16 = mybir.dt.bfloat16

B, S, D = 4, 2048, 1024
H, HD = 16, 64
DH = 512          # columns of the head-half handled by one core (8 heads * 64)
NCORES = 8
CH = 512          # q chunk width
NCH = S // CH     # 4
NKB = S // 128    # 16 key/query 128-blocks
KT = D // 128     # 8 contraction tiles for the projections
NWARM = 24        # PE warm-up matmuls issued while input DMAs land

_CACHED = None


def build_module():
    nc = bacc.Bacc("TRN2", target_bir_lowering=False, debug=False)

    # --- dram tensors: host-packed for few, large, contiguous DMAs --------
    # x chunk-major: xc{c}[p, 512*k + s] = X^T[128k+p, 512c+s]
    xcd = [nc.dram_tensor(f"xc{c}", [128, KT * CH], BF16, kind="ExternalInput")
           for c in range(NCH)]
    # wk/wq pair-major, split so pair 0 can arrive first:
    #   wk0[p, 128k+i] = Wk[128k+p, i]  (pair g=0)
    #   wkR[p, 1024(g-1) + 128k + i] = Wk[128k+p, 128g+i]  (pairs 1..3)
    wk0d = nc.dram_tensor("wk0", [128, KT * 128], BF16, kind="ExternalInput")
    wq0d = nc.dram_tensor("wq0", [128, KT * 128], BF16, kind="ExternalInput")
    wkRd = nc.dram_tensor("wkR", [128, 3 * KT * 128], BF16, kind="ExternalInput")
    wqRd = nc.dram_tensor("wqR", [128, 3 * KT * 128], BF16, kind="ExternalInput")
    # wv k-major: wv[p, 512k+j] = Wv[128k+p, j]
    wvd = nc.dram_tensor("wv", [128, KT * DH], BF16, kind="ExternalInput")
    # wo g-major: wo[p, 1024g+j] = Wo[128g+p, j]
    wod = nc.dram_tensor("wo", [128, 4 * D], BF16, kind="ExternalInput")
    maskt = nc.dram_tensor("maskt", [128, 128], BF16, kind="ExternalInput")
    # bf16 partials: halves the output DMA; the host sums the two half
    # partials in fp32 (quantization adds ~2e-3 rel err, well within budget)
    out = nc.dram_tensor("out", [S, D], BF16, kind="ExternalOutput")

    with tile.TileContext(nc) as tc, ExitStack() as ctx:
        const = ctx.enter_context(tc.tile_pool(name="const", bufs=1))
        xTp = ctx.enter_context(tc.tile_pool(name="xTp", bufs=1))
        wp = ctx.enter_context(tc.tile_pool(name="wp", bufs=1))
        qkp = ctx.enter_context(tc.tile_pool(name="qkp", bufs=1))
        vp = ctx.enter_context(tc.tile_pool(name="vp", bufs=1))
        ctp = ctx.enter_context(tc.tile_pool(name="ctp", bufs=1))
        pTp = ctx.enter_context(tc.tile_pool(name="pTp", bufs=4))
        rp = ctx.enter_context(tc.tile_pool(name="rp", bufs=3))
        stgp = ctx.enter_context(tc.tile_pool(name="stgp", bufs=4))
        bp = ctx.enter_context(tc.tile_pool(name="bp", bufs=4))
        osb = ctx.enter_context(tc.tile_pool(name="osb", bufs=2))
        ps_sc = ctx.enter_context(tc.tile_pool(name="ps_sc", bufs=2, space="PSUM"))
        ps_ctx = ctx.enter_context(tc.tile_pool(name="ps_ctx", bufs=2, space="PSUM"))
        ps_mm = ctx.enter_context(tc.tile_pool(name="ps_mm", bufs=2, space="PSUM"))

        # --- SBUF persistents --------------------------------------------
        xbig = xTp.tile([128, KT * S], BF16, name="xbig", tag="xbig")
        wk0 = wp.tile([128, KT * 128], BF16, name="wk0", tag="wk0")
        wq0 = wp.tile([128, KT * 128], BF16, name="wq0", tag="wq0")
        wkR = wp.tile([128, 3 * KT * 128], BF16, name="wkR", tag="wkR")
        wqR = wp.tile([128, 3 * KT * 128], BF16, name="wqR", tag="wqR")
        wv = wp.tile([128, KT * DH], BF16, name="wv", tag="wv")
        wo = wp.tile([128, 4 * D], BF16, name="wo", tag="wo")
        mask = const.tile([128, 128], BF16, name="mask", tag="mask")
        scr = const.tile([128, CH], BF16, name="scr", tag="scr")
        ones8 = const.tile([128, 8], BF16, name="ones8", tag="ones8")

        def xts(k):
            return xbig[:, S * k:S * (k + 1)]

        def wk_sl(g, k):
            if g == 0:
                return wk0[:, 128 * k:128 * (k + 1)]
            return wkR[:, 1024 * (g - 1) + 128 * k:1024 * (g - 1) + 128 * (k + 1)]

        def wq_sl(g, k):
            if g == 0:
                return wq0[:, 128 * k:128 * (k + 1)]
            return wqR[:, 1024 * (g - 1) + 128 * k:1024 * (g - 1) + 128 * (k + 1)]

        # --- input DMAs, priority order ----------------------------------
        nc.sync.dma_start(wk0[:], wk0d[:])
        nc.sync.dma_start(wq0[:], wq0d[:])
        xr = xbig[:].rearrange("p (k s) -> p k s", s=S)
        xcr0 = xcd[0][:].rearrange("p (k s) -> p k s", s=CH)
        nc.sync.dma_start(xr[:, 0:4, 0:CH], xcr0[:, 0:4, :])
        nc.sync.dma_start(xr[:, 4:KT, 0:CH], xcr0[:, 4:KT, :])
        nc.sync.dma_start(mask[:], maskt[:])
        nc.sync.dma_start(wv[:], wvd[:])
        for c in range(1, NCH):
            nc.sync.dma_start(xr[:, :, CH * c:CH * (c + 1)], xcd[c][:])
        nc.sync.dma_start(wkR[:], wkRd[:])
        nc.sync.dma_start(wqR[:], wqRd[:])
        nc.sync.dma_start(wo[:], wod[:])

        # --- PE warm-up: junk matmuls on a memset tile keep the PE busy
        # (and its p-state ramping) while the first inputs stream in.
        nc.vector.memset(scr[:], 0.0)
        nc.vector.memset(ones8[:], 1.0)
        wps = ps_mm.tile([128, CH], F32, name="mm", tag="mm")
        for _ in range(NWARM):
            nc.tensor.matmul(wps[:], lhsT=scr[:, 0:128], rhs=scr[:],
                             skip_group_check=True)

        # --- persistent intermediates ------------------------------------
        qts = [qkp.tile([128, S], BF16, name=f"qt{g}", tag=f"qt{g}") for g in range(4)]
        kts = [qkp.tile([128, S], BF16, name=f"kt{g}", tag=f"kt{g}") for g in range(4)]
        vts = [vp.tile([128, 8 * 65], BF16, name=f"v{m}", tag=f"v{m}") for m in range(NKB)]
        cts = [ctp.tile([128, S], BF16, name=f"ct{g}", tag=f"ct{g}") for g in range(4)]
        # the fused-ones columns of every V tile are constant: set them once
        for m in range(NKB):
            ones = vts[m][:].rearrange("p (h x) -> p h x", x=65)[:, :, 64:65]
            nc.vector.tensor_copy(ones, ones8[:])

        # --- filler units: closures over in-flight PSUM accumulations ----
        fillers = []
        fill_pos = 0

        def pop_fill(n):
            nonlocal fill_pos
            end = min(fill_pos + n, len(fillers))
            while fill_pos < end:
                fillers[fill_pos][1]()
                fill_pos += 1

        def drain_tag(tag):
            """Run every queued unit up to and including the last one with
            this tag (correctness-of-perf: deps are real semaphores)."""
            nonlocal fill_pos
            last = -1
            for i in range(fill_pos, len(fillers)):
                if fillers[i][0] == tag:
                    last = i
            while fill_pos <= last:
                fillers[fill_pos][1]()
                fill_pos += 1

        def push_qk_units(dst, wsl, g, c):
            st8 = {}

            def a():
                ps = ps_mm.tile([128, CH], F32, name="mm", tag="mm")
                st8["ps"] = ps
                for k in range(4):
                    nc.tensor.matmul(ps[:], lhsT=wsl(g, k),
                                     rhs=xts(k)[:, CH * c:CH * (c + 1)],
                                     start=(k == 0), stop=False,
                                     skip_group_check=True)

            def b():
                ps = st8["ps"]
                for k in range(4, KT):
                    nc.tensor.matmul(ps[:], lhsT=wsl(g, k),
                                     rhs=xts(k)[:, CH * c:CH * (c + 1)],
                                     start=False, stop=(k == KT - 1),
                                     skip_group_check=True)
                nc.vector.tensor_copy(dst[:, CH * c:CH * (c + 1)], ps[:])

            tag = f"qk{g}c{c}"
            fillers.append((tag, a))
            fillers.append((tag, b))

        def push_v_units(m):
            st8 = {}

            def a():
                ps = ps_mm.tile([128, CH], F32, name="mm", tag="mm")
                st8["ps"] = ps
                for k in range(4):
                    nc.tensor.matmul(ps[:], lhsT=xts(k)[:, 128 * m:128 * (m + 1)],
                                     rhs=wv[:, DH * k:DH * (k + 1)],
                                     start=(k == 0), stop=False,
                                     skip_group_check=True)

            def b():
                ps = st8["ps"]
                for k in range(4, KT):
                    nc.tensor.matmul(ps[:], lhsT=xts(k)[:, 128 * m:128 * (m + 1)],
                                     rhs=wv[:, DH * k:DH * (k + 1)],
                                     start=False, stop=(k == KT - 1),
                                     skip_group_check=True)
                vm = vts[m]
                dst = vm[:].rearrange("p (h x) -> p h x", x=65)[:, :, 0:64]
                src = ps[:].rearrange("p (h d) -> p h d", d=64)
                nc.vector.tensor_copy(dst, src)

            tag = f"v{m}"
            fillers.append((tag, a))
            fillers.append((tag, b))

        def push_outproj_units(c):
            for qb in range(4 * c, 4 * c + 4):
                st8 = {}

                def a(qb=qb, st8=st8):
                    ost = osb.tile([128, D], BF16, name="ost", tag="ost")
                    ps = ps_mm.tile([128, CH], F32, name="mm", tag="mm")
                    st8["ost"], st8["ps"] = ost, ps
                    for g in range(4):
                        nc.tensor.matmul(ps[:], lhsT=cts[g][:, 128 * qb:128 * (qb + 1)],
                                         rhs=wo[:, D * g:D * g + CH],
                                         start=(g == 0), stop=(g == 3),
                                         skip_group_check=True)
                    nc.vector.tensor_copy(ost[:, 0:CH], ps[:])

                def b(qb=qb, st8=st8):
                    ost = st8["ost"]
                    ps = ps_mm.tile([128, CH], F32, name="mm", tag="mm")
                    for g in range(4):
                        nc.tensor.matmul(ps[:], lhsT=cts[g][:, 128 * qb:128 * (qb + 1)],
                                         rhs=wo[:, D * g + CH:D * (g + 1)],
                                         start=(g == 0), stop=(g == 3),
                                         skip_group_check=True)
                    nc.vector.tensor_copy(ost[:, CH:D], ps[:])
                    nc.sync.dma_start(out[128 * qb:128 * (qb + 1), :], ost[:])

                tag = f"op{c}"
                fillers.append((tag, a))
                fillers.append((tag, b))

        # --- attention ----------------------------------------------------
        def run_chunk(g, c):
            """Heads (2g, 2g+1), queries [CH*c, CH*(c+1)); software-pipelined:
            sc(j+1) and filler units are issued before pv(j)."""
            drain_tag(f"qk{g}c{c}")
            qt, ktile = qts[g], kts[g]
            nj = 4 * c + 4
            ctx_e = ps_ctx.tile([65, CH], F32, name="ctx", tag="ctx")
            ctx_o = ps_ctx.tile([65, CH], F32, name="ctx", tag="ctx")
            pts = [None] * nj

            def sc_step(j):
                d = j - 4 * c
                st = 128 * max(0, d)
                sc = ps_sc.tile([128, 2 * CH], F32, name="sc", tag="sc")
                # even head in bank 0 (cols 0:512), odd head in bank 1
                nc.tensor.matmul(
                    sc[:, st:CH], lhsT=ktile[0:64, 128 * j:128 * (j + 1)],
                    rhs=qt[0:64, CH * c + st:CH * (c + 1)])
                nc.tensor.matmul(
                    sc[:, CH + st:2 * CH], lhsT=ktile[64:128, 128 * j:128 * (j + 1)],
                    rhs=qt[64:128, CH * c + st:CH * (c + 1)])
                pt = pTp.tile([128, 2 * CH], BF16, name="pT", tag="pT")
                if st == 0:
                    # one exp covers both heads
                    nc.scalar.activation(pt[:], sc[:],
                                         mybir.ActivationFunctionType.Exp,
                                         scale=0.125)
                else:
                    nc.scalar.activation(pt[:, st:CH], sc[:, st:CH],
                                         mybir.ActivationFunctionType.Exp,
                                         scale=0.125)
                    nc.scalar.activation(pt[:, CH + st:], sc[:, CH + st:],
                                         mybir.ActivationFunctionType.Exp,
                                         scale=0.125)
                if d >= 0:
                    # causal zeroing of the diagonal square, post-exp: a cheap
                    # bf16 SBUF multiply on DVE (GpSimd must stay single-ucode
                    # for partition_broadcast — mixing op kinds there forces
                    # ~6us LOAD_LIB swaps)
                    ms = slice(st, st + 128)
                    nc.vector.tensor_tensor(pt[:, ms], pt[:, ms], mask[:],
                                            op=mybir.AluOpType.mult)
                    ms2 = slice(CH + st, CH + st + 128)
                    nc.vector.tensor_tensor(pt[:, ms2], pt[:, ms2], mask[:],
                                            op=mybir.AluOpType.mult)
                pts[j] = (pt, st)

            def pv_step(j):
                pt, st = pts[j]
                he, ho = 2 * g, 2 * g + 1
                nc.tensor.matmul(ctx_e[:, st:], lhsT=vts[j][:, 65 * he:65 * he + 65],
                                 rhs=pt[:, st:CH], start=(j == 0), stop=(j == nj - 1),
                                 skip_group_check=True)
                nc.tensor.matmul(ctx_o[:, st:], lhsT=vts[j][:, 65 * ho:65 * ho + 65],
                                 rhs=pt[:, CH + st:2 * CH], start=(j == 0),
                                 stop=(j == nj - 1), skip_group_check=True)

            sc_step(0)
            for j in range(nj):
                if j + 1 < nj:
                    drain_tag(f"v{j + 1}")  # usually a no-op; keeps pv fed
                    pop_fill(1)  # spaces sc(j) and sc(j+1) PSUM-bank reuse
                    sc_step(j + 1)
                pop_fill(1)
                pv_step(j)

            # normalize by the fused denominator row and store into ctxT (bf16)
            for ctx_ps, rows in ((ctx_e, slice(0, 64)), (ctx_o, slice(64, 128))):
                # stage to SBUF first: frees the ctx PSUM bank immediately so
                # the next chunk's PV can reuse it without waiting on the
                # reciprocal/broadcast round trip
                stg = stgp.tile([65, CH], F32, name="stg", tag="stg")
                nc.vector.tensor_copy(stg[:], ctx_ps[:])
                # copy the sums row to partition 0 (both the custom-DVE recip
                # and the HW partition_broadcast need a partition-0 source),
                # take the reciprocal, and broadcast it across 64 partitions
                srow = rp.tile([1, CH], F32, name="srow", tag="srow")
                nc.vector.tensor_copy(srow[:], stg[64:65, :])
                rc = rp.tile([1, CH], F32, name="recip", tag="recip")
                nc.vector.reciprocal_approx_fast(rc[:], srow[:])
                bc = bp.tile([64, CH], F32, name="bcast", tag="bcast")
                nc.gpsimd.partition_broadcast(bc[:], rc[:])

                # the cts write is deferred into the filler queue (popped early
                # in the next chunk) so the DVE queue never blocks the next
                # chunk's mask -> exp -> PV chain on this chunk's normalize
                def mult(stg=stg, bc=bc, rows=rows, g=g, c=c):
                    nc.vector.tensor_tensor(cts[g][rows, CH * c:CH * (c + 1)],
                                            stg[0:64, :], bc[:],
                                            op=mybir.AluOpType.mult)
                fillers.insert(fill_pos, ("norm", mult))

        # --- schedule -----------------------------------------------------
        # pair-0 K/Q chunk 0 directly (first PE work, ~8us in)
        push_qk_units(kts[0], wk_sl, 0, 0)
        push_qk_units(qts[0], wq_sl, 0, 0)
        drain_tag("qk0c0")
        # filler queue for the g=0 phase: V blocks as needed + later chunks
        for m in range(4):
            push_v_units(m)
        for c in range(1, NCH):
            push_qk_units(kts[0], wk_sl, 0, c)
            push_qk_units(qts[0], wq_sl, 0, c)
            for m in range(4 * c, 4 * c + 4):
                push_v_units(m)
        # pair-1 projections also as g=0-phase filler
        for c in range(NCH):
            push_qk_units(kts[1], wk_sl, 1, c)
            push_qk_units(qts[1], wq_sl, 1, c)

        for g in range(4):
            if g >= 1:
                # next pair's projections pop as filler during this phase
                for c in range(NCH):
                    if g + 1 < 4:
                        push_qk_units(kts[g + 1], wk_sl, g + 1, c)
                        push_qk_units(qts[g + 1], wq_sl, g + 1, c)
            for c in range(NCH):
                run_chunk(g, c)
                if g == 3:
                    push_outproj_units(c)
        pop_fill(len(fillers))  # flush remaining out-proj units

    nc.compile()
    return nc


def _get_module():
    global _CACHED
    if _CACHED is None:
        _CACHED = build_module()
    return _CACHED


def _causal_mask_tile():
    k = np.arange(128)[:, None]
    q = np.arange(128)[None, :]
    return np.where(k <= q, 1.0, 0.0).astype(ml_dtypes.bfloat16)


def make_in_maps(inputs, Wq, Wk, Wv, Wo):
    """Host-side packing into the dram-tensor layouts (see build_module)."""
    bf = ml_dtypes.bfloat16
    mask = _causal_mask_tile()
    in_maps = []
    for core in range(NCORES):
        b, hh = core // 2, core % 2
        cols = slice(DH * hh, DH * (hh + 1))
        wqh = Wq[:, cols].reshape(KT, 128, 4, 128)   # [k,p,g,i]
        wkh = Wk[:, cols].reshape(KT, 128, 4, 128)
        wvh = Wv[:, cols].reshape(KT, 128, DH)       # [k,p,j]
        woh = Wo[cols, :].reshape(4, 128, D)         # [g,p,j]
        xT = inputs[b].T.reshape(KT, 128, NCH, CH)   # [k,p,c,s]
        m = {
            "wk0": wkh[:, :, 0, :].transpose(1, 0, 2).reshape(128, KT * 128),
            "wq0": wqh[:, :, 0, :].transpose(1, 0, 2).reshape(128, KT * 128),
            "wkR": wkh[:, :, 1:, :].transpose(1, 2, 0, 3).reshape(128, 3 * KT * 128),
            "wqR": wqh[:, :, 1:, :].transpose(1, 2, 0, 3).reshape(128, 3 * KT * 128),
            "wv": wvh.transpose(1, 0, 2).reshape(128, KT * DH),
            "wo": woh.transpose(1, 0, 2).reshape(128, 4 * D),
        }
        for c in range(NCH):
            m[f"xc{c}"] = xT[:, :, c, :].transpose(1, 0, 2).reshape(128, KT * CH)
        m = {k: np.ascontiguousarray(v).astype(bf) for k, v in m.items()}
        m["maskt"] = mask
        in_maps.append(m)
    return in_maps


def kernel(inputs, Wq, Wk, Wv, Wo, bo):
    inputs = np.asarray(inputs, dtype=np.float32)
    Wq = np.asarray(Wq, dtype=np.float32)
    Wk = np.asarray(Wk, dtype=np.float32)
    Wv = np.asarray(Wv, dtype=np.float32)
    Wo = np.asarray(Wo, dtype=np.float32)
    bo = np.asarray(bo, dtype=np.float32)

    in_maps = make_in_maps(inputs, Wq, Wk, Wv, Wo)
    nc = _get_module()
    res = bass_utils.run_bass_kernel_spmd(nc, in_maps, core_ids=list(range(NCORES)))
    outs = [r["out"] for r in res.results]

    full = np.empty((B, S, D), dtype=np.float32)
    for b in range(B):
        full[b] = (outs[2 * b].astype(np.float32)
                   + outs[2 * b + 1].astype(np.float32) + bo[None, :])
    return full
